# revision 1
# baseline (speedup 1.0000x reference)
"""Trainium2 multi-head attention kernel (8 NeuronCores).

Sharding: 2 (batch) x 4 (head-group) grid. Core c handles batch b=c//4 and
heads [4g, 4g+4) where g=c%4 (d_model slice of 256).

Per core:
  1. Q^T,K^T projections [256,2048] (fp32r) and V [2048,256] (fp16) for its
     heads, contraction d_model=1024.
  2. Attention: scores^T = Kh^T.T @ Qh^T per head (two heads packed onto PE
     row-groups via tile_position), exp via ScalarE with fused 1/8 scale, AV
     in fp16 with a ones-augmented V so the softmax denominators fall out of
     the same matmul, then an accurate-reciprocal normalize.
  3. Partial output projection, computed transposed (out^T = WoT.T @ attn^T)
     so the weight stays stationary: [1024, 2048] partial sum in fp16.
Host: all inputs are pre-transposed/sliced per core; the 4 partial outputs of
each batch are summed on host (the unshard step of this tensor-parallel
layout), transposed back and concatenated over batch.
"""
import os
import sys

import numpy as np

for _p in ("/opt/trn_rl_repo", "/root/.axon_site/_ro/trn_rl_repo"):
    if _p not in sys.path:
        sys.path.append(_p)

import concourse.bacc as bacc
import concourse.mybir as mybir
import concourse.tile as tile
from concourse.bass_utils import run_bass_kernel_spmd

F32 = mybir.dt.float32
F32R = mybir.dt.float32r
F16 = mybir.dt.float16

B, S, D, H, DK = 2, 2048, 1024, 16, 64
NC_ = 8
HG = D // 4          # 256: d_model slice per core
KT_D = D // 128      # 8 contraction tiles for projections
KT_S = S // 128      # 16 sequence tiles
QC = S // 512        # 4 query chunks of 512
AF = mybir.ActivationFunctionType

FP16_SCORES = os.environ.get("FP16_SCORES") == "1"
SCORE_DT = F16 if FP16_SCORES else F32R

if os.environ.get("LDW_OPT") == "1":
    import concourse.bass_utils as _bu

    if not getattr(_bu, "_ldw_opt_patched", False):
        _orig_run_command = _bu.run_command

        def _run_command_ldw(cmd, **kw):
            cmd = [c.replace("--enable-ldw-opt=false", "--enable-ldw-opt=true")
                   if isinstance(c, str) else c for c in cmd]
            return _orig_run_command(cmd, **kw)

        _bu.run_command = _run_command_ldw
        _bu._ldw_opt_patched = True


def build_nc():
    nc = bacc.Bacc("TRN2", target_bir_lowering=False, debug=False, num_devices=NC_)

    xqT = nc.dram_tensor("xqT", [D, S], F32R, kind="ExternalInput").ap()
    xkT = nc.dram_tensor("xkT", [D, S], F32R, kind="ExternalInput").ap()
    xvT = nc.dram_tensor("xvT", [D, S], F16, kind="ExternalInput").ap()
    wqT = nc.dram_tensor("wqT", [D, HG], F32R, kind="ExternalInput").ap()
    wkT = nc.dram_tensor("wkT", [D, HG], F32R, kind="ExternalInput").ap()
    wvT = nc.dram_tensor("wvT", [D, HG], F16, kind="ExternalInput").ap()
    bqv = nc.dram_tensor("bqv", [128, 4], F32, kind="ExternalInput").ap()
    bvb = nc.dram_tensor("bvb", [128, HG], F32, kind="ExternalInput").ap()
    woT = nc.dram_tensor("woT", [HG, D], F16, kind="ExternalInput").ap()
    bob = nc.dram_tensor("bob", [128, KT_D], F32, kind="ExternalInput").ap()
    outT = nc.dram_tensor("outT", [D, S], F16, kind="ExternalOutput").ap()

    with tile.TileContext(nc) as tc:
        with (
            tc.tile_pool(name="const", bufs=1) as cpool,
            tc.tile_pool(name="proj", bufs=1) as ppool,
            tc.tile_pool(name="xs", bufs=4) as xpool,
            tc.tile_pool(name="exp", bufs=4) as epool,
            tc.tile_pool(name="nrm", bufs=4) as npool,
            tc.tile_pool(name="ost", bufs=4) as opool,
            tc.tile_pool(name="psC", bufs=1, space="PSUM") as psC,
        ):
            # ---- persistent tiles ----
            wq_t = cpool.tile([128, KT_D, HG], F32R)
            wk_t = cpool.tile([128, KT_D, HG], F32R)
            wv_t = cpool.tile([128, KT_D, HG], F16)
            bqv_t = cpool.tile([128, 4], F32)
            bvb_t = cpool.tile([128, HG], F32)
            bob_t = cpool.tile([128, KT_D], F32)
            wo_t = cpool.tile([128, 2, D], F16)

            qT = ppool.tile([128, 2, S], SCORE_DT)  # [o-part, Mtile, t]
            kT = ppool.tile([128, 2, S], SCORE_DT)
            vS = ppool.tile([128, KT_S, 4 * 128], F16)  # [t-part, t-tile, head*65]
            aoT = ppool.tile([128, 2, S], F16)  # normalized attn out^T

            # ---- phase A: Q^T then K^T projections ----
            # 4 accumulators live (2 Mtiles x 2 qc of a pair), borrowed from
            # the attention pool's tags; lhsT reused across the qc pair.
            for (w_t, w_d, dst, xsrc, xtag, boff) in (
                (wq_t, wqT, qT, xqT, "xq_c", 0),
                (wk_t, wkT, kT, xkT, "xk_c", 2),
            ):
                for kt in range(KT_D):
                    nc.sync.dma_start(
                        w_t[:, kt, :], w_d[kt * 128:(kt + 1) * 128, :])
                if boff == 0:
                    nc.sync.dma_start(bqv_t[:], bqv[:])
                for qp in range(QC // 2):
                    acc = [
                        psC.tile([128, 512], F32, name=f"acc{m}{j}",
                                 tag=f"av{j}", bufs=2)
                        for m in range(2) for j in range(2)
                    ]
                    for kt in range(KT_D):
                        x_c = xpool.tile([128, 1024], F32R, name="x_c",
                                         tag=xtag)
                        nc.sync.dma_start(
                            x_c[:], xsrc[kt * 128:(kt + 1) * 128,
                                         qp * 1024:(qp + 1) * 1024])
                        for m in range(2):
                            for j in range(2):
                                nc.tensor.matmul(
                                    acc[m * 2 + j][:],
                                    w_t[:, kt, m * 128:(m + 1) * 128],
                                    x_c[:, j * 512:(j + 1) * 512],
                                    start=(kt == 0), stop=(kt == KT_D - 1))
                    for m in range(2):
                        for j in range(2):
                            qc = qp * 2 + j
                            nc.scalar.activation(
                                dst[:, m, qc * 512:(qc + 1) * 512],
                                acc[m * 2 + j][:],
                                AF.Identity,
                                bias=bqv_t[:, boff + m:boff + m + 1])

            # ---- phase B: V projection (token-major layout, fp16) ----
            # ones columns of vS (softmax denominator trick): fill whole
            # tile with 1.0 once; V writes below overwrite all but col 64.
            nc.gpsimd.memset(vS[:], 1.0)
            for kt in range(KT_D):
                nc.sync.dma_start(
                    wv_t[:, kt, :], wvT[kt * 128:(kt + 1) * 128, :])
            nc.sync.dma_start(bvb_t[:], bvb[:])
            # one accumulator per PSUM bank (interleaved chains in a single
            # bank corrupt each other: start=True clears the whole bank)
            for tc4 in range(KT_S // 4):
                psv = [psC.tile([128, HG], F32, name=f"psv{t}",
                                tag=f"av{t % 2}", bufs=2) for t in range(4)]
                for kt in range(KT_D):
                    xv_c = xpool.tile([128, 512], F16, name="xv_c", tag="xv_c")
                    nc.sync.dma_start(
                        xv_c[:], xvT[kt * 128:(kt + 1) * 128,
                                     tc4 * 512:(tc4 + 1) * 512])
                    for t in range(4):
                        nc.tensor.matmul(
                            psv[t][:],
                            xv_c[:, t * 128:(t + 1) * 128],
                            wv_t[:, kt, :], start=(kt == 0),
                            stop=(kt == KT_D - 1))
                for t in range(4):
                    tg = tc4 * 4 + t
                    for h in range(4):
                        nc.vector.tensor_tensor(
                            vS[:, tg, h * 128:h * 128 + 64],
                            psv[t][:, h * 64:(h + 1) * 64],
                            bvb_t[:, h * 64:(h + 1) * 64],
                            op=mybir.AluOpType.add)

            # ---- phase C: attention ----
            for p in range(2):
                for qc in range(QC):
                    av = [psC.tile([128, 512], F32, name=f"av{i}",
                                   tag=f"av{i}", bufs=2) for i in range(2)]
                    exs = []

                    def av_mms(kt):
                        for i in range(2):
                            nc.tensor.matmul(
                                av[i][:],
                                vS[:, kt, (2 * p + i) * 128:
                                   (2 * p + i + 1) * 128],
                                exs[kt][:, i * 512:(i + 1) * 512],
                                start=(kt == 0), stop=(kt == KT_S - 1))

                    for kt in range(KT_S):
                        sc = psC.tile([128, 1024], F32, name="sc", tag="sc",
                                      bufs=2)
                        nc.tensor.matmul(
                            sc[:, 0:512],
                            kT[0:64, p, kt * 128:(kt + 1) * 128],
                            qT[0:64, p, qc * 512:(qc + 1) * 512],
                            start=True, stop=True, tile_position=(0, 0))
                        nc.tensor.matmul(
                            sc[:, 512:1024],
                            kT[64:128, p, kt * 128:(kt + 1) * 128],
                            qT[64:128, p, qc * 512:(qc + 1) * 512],
                            start=True, stop=True, tile_position=(64, 0))
                        ex = epool.tile([128, 1024], F16, name="ex", tag="ex")
                        nc.scalar.activation(ex[:], sc[:], AF.Exp, scale=0.125)
                        exs.append(ex)
                        # AV lags two kt so exp(k) has a full PE cycle of
                        # cover before its consumers issue
                        if kt > 1:
                            av_mms(kt - 2)
                    av_mms(KT_S - 2)
                    av_mms(KT_S - 1)
                    for i in range(2):
                        sr = npool.tile([1, 512], F32, name="sr", tag=f"sr{i}")
                        nc.vector.tensor_copy(sr[:], av[i][64:65, :])
                        rc = npool.tile([1, 512], F32, name="rc", tag=f"rc{i}")
                        scr = npool.tile([1, 512], F32, name="scr", tag=f"scr{i}")
                        nc.vector.reciprocal_approx_accurate(rc[:], sr[:], scr[:])
                        rb = npool.tile([64, 512], F32, name="rb", tag=f"rb{i}")
                        nc.gpsimd.partition_broadcast(rb[:], rc[:])
                        nc.vector.tensor_tensor(
                            aoT[i * 64:(i + 1) * 64, p,
                                qc * 512:(qc + 1) * 512],
                            av[i][0:64, :], rb[:], op=mybir.AluOpType.mult)

            for k2 in range(2):
                nc.sync.dma_start(
                    wo_t[:, k2, :], woT[k2 * 128:(k2 + 1) * 128, :])
            nc.sync.dma_start(bob_t[:], bob[:])

            # ---- phase D: partial output projection, transposed ----
            # out^T[o, t] = sum_d WoT[d, o] * attn^T[d, t]; Wo stays
            # stationary across the 4 token chunks.
            for ot in range(KT_D):
                acc2 = [
                    psC.tile([128, 512], F32, name=f"acc2{tcx}",
                             tag=("sc" if tcx < 2 else f"av{tcx - 2}"),
                             bufs=2)
                    for tcx in range(4)
                ]
                for k2 in range(2):
                    for tcx in range(4):
                        nc.tensor.matmul(
                            acc2[tcx][:],
                            wo_t[:, k2, ot * 128:(ot + 1) * 128],
                            aoT[:, k2, tcx * 512:(tcx + 1) * 512],
                            start=(k2 == 0), stop=(k2 == 1))
                for tcx in range(4):
                    o_st = opool.tile([128, 512], F16, name="o_st", tag="o_st")
                    nc.vector.tensor_scalar_add(
                        o_st[:], acc2[tcx][:], bob_t[:, ot:ot + 1])
                    nc.sync.dma_start(
                        outT[ot * 128:(ot + 1) * 128,
                             tcx * 512:(tcx + 1) * 512], o_st[:])

    nc.compile()
    return nc


_NC = None


def _get_nc():
    global _NC
    if _NC is None:
        _NC = build_nc()
    return _NC


def kernel(q, k, v, Wq, bq, Wk, bk, Wv, bv, Wo, bo):
    nc = _get_nc()

    q = np.asarray(q, np.float32)
    k = np.asarray(k, np.float32)
    v = np.asarray(v, np.float32)

    xT = {}
    for b in range(B):
        xT[("q", b)] = np.ascontiguousarray(q[b].T)
        xT[("k", b)] = np.ascontiguousarray(k[b].T)
        xT[("v", b)] = np.ascontiguousarray(v[b].T).astype(np.float16)

    WqT = np.ascontiguousarray(np.asarray(Wq, np.float32).T)
    WkT = np.ascontiguousarray(np.asarray(Wk, np.float32).T)
    WvT = np.asarray(Wv, np.float32).T.astype(np.float16)
    WoT = np.asarray(Wo, np.float32).T.astype(np.float16)
    bq = np.asarray(bq, np.float32)
    bk = np.asarray(bk, np.float32)
    bv = np.asarray(bv, np.float32)
    bo = np.asarray(bo, np.float32)

    in_maps = []
    for c in range(NC_):
        b, g = divmod(c, 4)
        sl = slice(g * HG, (g + 1) * HG)
        bqs, bks = bq[sl], bk[sl]
        bqv_a = np.stack(
            [bqs[0:128], bqs[128:256], bks[0:128], bks[128:256]], axis=1)
        bo_a = (bo if g == 0 else np.zeros_like(bo)).reshape(KT_D, 128).T
        in_maps.append({
            "xqT": xT[("q", b)],
            "xkT": xT[("k", b)],
            "xvT": xT[("v", b)],
            "wqT": np.ascontiguousarray(WqT[:, sl]),
            "wkT": np.ascontiguousarray(WkT[:, sl]),
            "wvT": np.ascontiguousarray(WvT[:, sl]),
            "bqv": np.ascontiguousarray(bqv_a),
            "bvb": np.ascontiguousarray(
                np.broadcast_to(bv[sl], (128, HG))),
            "woT": np.ascontiguousarray(WoT[sl, :]),
            "bob": np.ascontiguousarray(bo_a),
        })

    res = run_bass_kernel_spmd(nc, in_maps, list(range(NC_)))

    out = np.empty((B, S, D), np.float32)
    for b in range(B):
        acc = np.zeros((D, S), np.float32)
        for g in range(4):
            acc += res.results[b * 4 + g]["outT"].astype(np.float32)
        out[b] = acc.T
    return out



# revision 2
# speedup vs baseline: 1.0256x; 1.0256x over previous
"""Trainium2 multi-head attention kernel (8 NeuronCores).

Sharding: 2 (batch) x 4 (head-group) grid. Core c handles batch b=c//4 and
heads [4g, 4g+4) where g=c%4 (d_model slice of 256).

All-fp16 datapath (PSUM accumulation stays fp32): inputs, weights, scores,
attention probabilities. Phase C is paced by the ScalarE exp stream
(1016ns per [128,1024] tile); the fp16 score MM pairs (row-packed via
tile_position) and AV MMs leave ~350ns/kt of PE slack, into which the
output projection for the previous token chunk is interleaved.

Per core:
  A. Q^T,K^T projections [256,2048] for its heads (fp16, contraction 1024).
  B. V [2048,256] token-major with a ones column (softmax denominators fall
     out of the AV matmul).
  C. Attention per (qc, p): scores^T pairs -> exp (ScalarE, fused 1/8
     scale) -> AV in fp16, then accurate-reciprocal normalize into aoT.
     After both p of a token chunk, the partial output projection
     out^T = WoT.T @ attn^T for that chunk is emitted into the next
     iteration's exp-paced slack.
Host: inputs pre-transposed/sliced per core; the 4 partial outputs of each
batch are summed on host, transposed back and concatenated over batch.
"""
import sys

import numpy as np

for _p in ("/opt/trn_rl_repo", "/root/.axon_site/_ro/trn_rl_repo"):
    if _p not in sys.path:
        sys.path.append(_p)

import concourse.bacc as bacc
import concourse.mybir as mybir
import concourse.tile as tile
from concourse.bass_utils import run_bass_kernel_spmd

F32 = mybir.dt.float32
F16 = mybir.dt.float16

B, S, D, H, DK = 2, 2048, 1024, 16, 64
NC_ = 8
HG = D // 4          # 256: d_model slice per core
KT_D = D // 128      # 8 contraction tiles for projections
KT_S = S // 128      # 16 sequence tiles
QC = S // 512        # 4 query chunks of 512
AF = mybir.ActivationFunctionType


def build_nc():
    nc = bacc.Bacc("TRN2", target_bir_lowering=False, debug=False, num_devices=NC_)

    xqT = nc.dram_tensor("xqT", [D, S], F16, kind="ExternalInput").ap()
    xkT = nc.dram_tensor("xkT", [D, S], F16, kind="ExternalInput").ap()
    xvT = nc.dram_tensor("xvT", [D, S], F16, kind="ExternalInput").ap()
    wqT = nc.dram_tensor("wqT", [D, HG], F16, kind="ExternalInput").ap()
    wkT = nc.dram_tensor("wkT", [D, HG], F16, kind="ExternalInput").ap()
    wvT = nc.dram_tensor("wvT", [D, HG], F16, kind="ExternalInput").ap()
    bqv = nc.dram_tensor("bqv", [128, 4], F32, kind="ExternalInput").ap()
    bvb = nc.dram_tensor("bvb", [128, HG], F32, kind="ExternalInput").ap()
    woT = nc.dram_tensor("woT", [HG, D], F16, kind="ExternalInput").ap()
    bob = nc.dram_tensor("bob", [128, KT_D], F32, kind="ExternalInput").ap()
    outT = nc.dram_tensor("outT", [D, S], F16, kind="ExternalOutput").ap()

    with tile.TileContext(nc) as tc:
        with (
            tc.tile_pool(name="const", bufs=1) as cpool,
            tc.tile_pool(name="proj", bufs=1) as ppool,
            tc.tile_pool(name="xs", bufs=4) as xpool,
            tc.tile_pool(name="exp", bufs=8) as epool,
            tc.tile_pool(name="nrm", bufs=4) as npool,
            tc.tile_pool(name="ost", bufs=4) as opool,
            tc.tile_pool(name="psC", bufs=1, space="PSUM") as psC,
        ):
            # ---- persistent tiles ----
            wq_t = cpool.tile([128, KT_D, HG], F16)
            wk_t = cpool.tile([128, KT_D, HG], F16)
            wv_t = cpool.tile([128, KT_D, HG], F16)
            bqv_t = cpool.tile([128, 4], F32)
            bvb_t = cpool.tile([128, HG], F32)
            bob_t = cpool.tile([128, KT_D], F32)
            wo_t = cpool.tile([128, 2, D], F16)

            qT = ppool.tile([128, 2, S], F16)  # [o-part, Mtile, t]
            kT = ppool.tile([128, 2, S], F16)
            vS = ppool.tile([128, KT_S, 4 * 128], F16)  # [t-part, t-tile, head*65]
            aoT = ppool.tile([128, 2, S], F16)  # normalized attn out^T

            # prefetch all weights up front so phase transitions never wait
            for kt in range(KT_D):
                nc.sync.dma_start(wq_t[:, kt, :], wqT[kt * 128:(kt + 1) * 128, :])
            nc.sync.dma_start(bqv_t[:], bqv[:])
            for kt in range(KT_D):
                nc.sync.dma_start(wk_t[:, kt, :], wkT[kt * 128:(kt + 1) * 128, :])
            for kt in range(KT_D):
                nc.sync.dma_start(wv_t[:, kt, :], wvT[kt * 128:(kt + 1) * 128, :])
            nc.sync.dma_start(bvb_t[:], bvb[:])
            for k2 in range(2):
                nc.sync.dma_start(wo_t[:, k2, :], woT[k2 * 128:(k2 + 1) * 128, :])
            nc.sync.dma_start(bob_t[:], bob[:])

            # ---- phase A: Q^T then K^T projections (fp16) ----
            for (w_t, dst, xsrc, xtag, boff) in (
                (wq_t, qT, xqT, "xq_c", 0),
                (wk_t, kT, xkT, "xk_c", 2),
            ):
                for qp in range(QC // 2):
                    acc = [
                        psC.tile([128, 512], F32, name=f"acc{m}{j}",
                                 tag=f"av{j}", bufs=2)
                        for m in range(2) for j in range(2)
                    ]
                    for kt in range(KT_D):
                        x_c = xpool.tile([128, 1024], F16, name="x_c",
                                         tag=xtag)
                        nc.sync.dma_start(
                            x_c[:], xsrc[kt * 128:(kt + 1) * 128,
                                         qp * 1024:(qp + 1) * 1024])
                        for m in range(2):
                            for j in range(2):
                                nc.tensor.matmul(
                                    acc[m * 2 + j][:],
                                    w_t[:, kt, m * 128:(m + 1) * 128],
                                    x_c[:, j * 512:(j + 1) * 512],
                                    start=(kt == 0), stop=(kt == KT_D - 1))
                    for m in range(2):
                        for j in range(2):
                            qc = qp * 2 + j
                            nc.scalar.activation(
                                dst[:, m, qc * 512:(qc + 1) * 512],
                                acc[m * 2 + j][:],
                                AF.Identity,
                                bias=bqv_t[:, boff + m:boff + m + 1])

            # ---- phase B: V projection (token-major layout, fp16) ----
            # ones columns of vS (softmax denominator trick): fill whole
            # tile with 1.0 once; V writes below overwrite all but col 64.
            nc.gpsimd.memset(vS[:], 1.0)
            # one accumulator per PSUM bank (interleaved chains in a single
            # bank corrupt each other: start=True clears the whole bank)
            for tc4 in range(KT_S // 4):
                psv = [psC.tile([128, HG], F32, name=f"psv{t}",
                                tag=f"av{t % 2}", bufs=2) for t in range(4)]
                for kt in range(KT_D):
                    xv_c = xpool.tile([128, 512], F16, name="xv_c", tag="xv_c")
                    nc.sync.dma_start(
                        xv_c[:], xvT[kt * 128:(kt + 1) * 128,
                                     tc4 * 512:(tc4 + 1) * 512])
                    for t in range(4):
                        nc.tensor.matmul(
                            psv[t][:],
                            xv_c[:, t * 128:(t + 1) * 128],
                            wv_t[:, kt, :], start=(kt == 0),
                            stop=(kt == KT_D - 1))
                for t in range(4):
                    tg = tc4 * 4 + t
                    for h in range(4):
                        nc.vector.tensor_tensor(
                            vS[:, tg, h * 128:h * 128 + 64],
                            psv[t][:, h * 64:(h + 1) * 64],
                            bvb_t[:, h * 64:(h + 1) * 64],
                            op=mybir.AluOpType.add)

            # ---- phase C: attention, with phase D (output projection)
            # for the previous token chunk interleaved into the exp-paced
            # slack of each qc's first head-group iteration ----
            def emit_D(qc):
                for ot in range(KT_D):
                    acc2 = psC.tile([128, 512], F32, name=f"acc2{ot}",
                                    tag=f"av{ot % 2}", bufs=2)
                    for k2 in range(2):
                        nc.tensor.matmul(
                            acc2[:],
                            wo_t[:, k2, ot * 128:(ot + 1) * 128],
                            aoT[:, k2, qc * 512:(qc + 1) * 512],
                            start=(k2 == 0), stop=(k2 == 1))
                    o_st = opool.tile([128, 512], F16, name="o_st", tag="o_st")
                    nc.vector.tensor_scalar_add(
                        o_st[:], acc2[:], bob_t[:, ot:ot + 1])
                    nc.sync.dma_start(
                        outT[ot * 128:(ot + 1) * 128,
                             qc * 512:(qc + 1) * 512], o_st[:])

            for qc in range(QC):
                for p in range(2):
                    av = [None, None]
                    exs = []

                    def av_mms(kt, av=av):
                        for i in range(2):
                            nc.tensor.matmul(
                                av[i][:],
                                vS[:, kt, (2 * p + i) * 128:
                                   (2 * p + i + 1) * 128],
                                exs[kt][:, i * 512:(i + 1) * 512],
                                start=(kt == 0), stop=(kt == KT_S - 1))

                    for kt in range(KT_S):
                        sc = psC.tile([128, 1024], F32, name="sc", tag="sc",
                                      bufs=2)
                        nc.tensor.matmul(
                            sc[:, 0:512],
                            kT[0:64, p, kt * 128:(kt + 1) * 128],
                            qT[0:64, p, qc * 512:(qc + 1) * 512],
                            start=True, stop=True, tile_position=(0, 0))
                        nc.tensor.matmul(
                            sc[:, 512:1024],
                            kT[64:128, p, kt * 128:(kt + 1) * 128],
                            qT[64:128, p, qc * 512:(qc + 1) * 512],
                            start=True, stop=True, tile_position=(64, 0))
                        ex = epool.tile([128, 1024], F16, name="ex", tag="ex")
                        nc.scalar.activation(ex[:], sc[:], AF.Exp, scale=0.125)
                        exs.append(ex)
                        # interleave the previous chunk's output projection
                        # after the first two exp tiles are queued: the 16 D
                        # matmuls run on the PE while ScalarE chews exp
                        if kt == 1 and p == 0 and qc > 0:
                            emit_D(qc - 1)
                        if kt == 1:
                            # allocate AV accumulators after any D allocs so
                            # the av0/av1 rings stay deadlock-free
                            av[0] = psC.tile([128, 512], F32, name="avA",
                                             tag="av0", bufs=2)
                            av[1] = psC.tile([128, 512], F32, name="avB",
                                             tag="av1", bufs=2)
                        # AV lags two kt so exp(k) has a full PE cycle of
                        # cover before its consumers issue
                        if kt > 1:
                            av_mms(kt - 2)
                    av_mms(KT_S - 2)
                    av_mms(KT_S - 1)
                    for i in range(2):
                        sr = npool.tile([1, 512], F32, name="sr", tag=f"sr{i}")
                        nc.vector.tensor_copy(sr[:], av[i][64:65, :])
                        rc = npool.tile([1, 512], F32, name="rc", tag=f"rc{i}")
                        scr = npool.tile([1, 512], F32, name="scr", tag=f"scr{i}")
                        nc.vector.reciprocal_approx_accurate(rc[:], sr[:], scr[:])
                        rb = npool.tile([64, 512], F32, name="rb", tag=f"rb{i}")
                        nc.gpsimd.partition_broadcast(rb[:], rc[:])
                        nc.vector.tensor_tensor(
                            aoT[i * 64:(i + 1) * 64, p,
                                qc * 512:(qc + 1) * 512],
                            av[i][0:64, :], rb[:], op=mybir.AluOpType.mult)

            emit_D(QC - 1)

    nc.compile()
    return nc


_NC = None


def _get_nc():
    global _NC
    if _NC is None:
        _NC = build_nc()
    return _NC


def kernel(q, k, v, Wq, bq, Wk, bk, Wv, bv, Wo, bo):
    nc = _get_nc()

    q = np.asarray(q, np.float32)
    k = np.asarray(k, np.float32)
    v = np.asarray(v, np.float32)

    xT = {}
    for b in range(B):
        xT[("q", b)] = np.ascontiguousarray(q[b].T).astype(np.float16)
        xT[("k", b)] = np.ascontiguousarray(k[b].T).astype(np.float16)
        xT[("v", b)] = np.ascontiguousarray(v[b].T).astype(np.float16)

    WqT = np.asarray(Wq, np.float32).T.astype(np.float16)
    WkT = np.asarray(Wk, np.float32).T.astype(np.float16)
    WvT = np.asarray(Wv, np.float32).T.astype(np.float16)
    WoT = np.asarray(Wo, np.float32).T.astype(np.float16)
    bq = np.asarray(bq, np.float32)
    bk = np.asarray(bk, np.float32)
    bv = np.asarray(bv, np.float32)
    bo = np.asarray(bo, np.float32)

    in_maps = []
    for c in range(NC_):
        b, g = divmod(c, 4)
        sl = slice(g * HG, (g + 1) * HG)
        bqs, bks = bq[sl], bk[sl]
        bqv_a = np.stack(
            [bqs[0:128], bqs[128:256], bks[0:128], bks[128:256]], axis=1)
        bo_a = (bo if g == 0 else np.zeros_like(bo)).reshape(KT_D, 128).T
        in_maps.append({
            "xqT": xT[("q", b)],
            "xkT": xT[("k", b)],
            "xvT": xT[("v", b)],
            "wqT": np.ascontiguousarray(WqT[:, sl]),
            "wkT": np.ascontiguousarray(WkT[:, sl]),
            "wvT": np.ascontiguousarray(WvT[:, sl]),
            "bqv": np.ascontiguousarray(bqv_a),
            "bvb": np.ascontiguousarray(
                np.broadcast_to(bv[sl], (128, HG))),
            "woT": np.ascontiguousarray(WoT[sl, :]),
            "bob": np.ascontiguousarray(bo_a),
        })

    res = run_bass_kernel_spmd(nc, in_maps, list(range(NC_)))

    out = np.empty((B, S, D), np.float32)
    for b in range(B):
        acc = np.zeros((D, S), np.float32)
        for g in range(4):
            acc += res.results[b * 4 + g]["outT"].astype(np.float32)
        out[b] = acc.T
    return out


# revision 7
# speedup vs baseline: 1.2348x; 1.2040x over previous
"""Trainium2 multi-head attention kernel (8 NeuronCores).

Sharding: 2 (batch) x 4 (head-group) grid. Core c handles batch b=c//4 and
heads [4g, 4g+4) where g=c%4 (d_model slice of 256).

All-fp16 datapath (PSUM accumulates fp32). Phase C is paced by the ScalarE
exp stream (~1016ns per [128,1024] tile); fp16 score pairs (row-packed via
tile_position) and AV matmuls leave PE slack into which the output
projection of the previous token chunk is interleaved. All DMAs are 0.5-2MB
with per-partition-contiguous layouts (pre-swizzled on host) — small or
strided transfers run far below HBM bandwidth.

PSUM plan (8 banks): sc [128,1024] x2 bufs (4) + av0/av1 [128,512] x1 (2)
+ dD [128,512] x2 (2). Phase A runs 8 parallel accumulation chains across
all of them; phase B rotates its 4 accumulators through av0/av1/dD.

Normalize does an early [65,512] PSUM->SBUF copy so the AV bank frees one
exp-cycle after the last AV matmul, which is what lets av run single-
buffered.
"""
import sys

import numpy as np

for _p in ("/opt/trn_rl_repo", "/root/.axon_site/_ro/trn_rl_repo"):
    if _p not in sys.path:
        sys.path.append(_p)

import concourse.bacc as bacc
import concourse.mybir as mybir
import concourse.tile as tile
from concourse.bass_utils import run_bass_kernel_spmd

F32 = mybir.dt.float32
F16 = mybir.dt.float16

B, S, D, H, DK = 2, 2048, 1024, 16, 64
NC_ = 8
HG = D // 4          # 256: d_model slice per core
KT_D = D // 128      # 8 contraction tiles for projections
KT_S = S // 128      # 16 sequence tiles
QC = S // 512        # 4 query chunks of 512
AF = mybir.ActivationFunctionType


def build_nc():
    nc = bacc.Bacc("TRN2", target_bir_lowering=False, debug=False, num_devices=NC_)

    # all inputs pre-swizzled on host to partition-major contiguous layouts
    xqT = nc.dram_tensor("xqT", [4 * 128, 2 * S], F16, kind="ExternalInput").ap()
    xkT = nc.dram_tensor("xkT", [4 * 128, 2 * S], F16, kind="ExternalInput").ap()
    xvT = nc.dram_tensor("xvT", [2 * 128, 4 * S], F16, kind="ExternalInput").ap()
    wqT = nc.dram_tensor("wqT", [128, KT_D * HG], F16, kind="ExternalInput").ap()
    wkT = nc.dram_tensor("wkT", [128, KT_D * HG], F16, kind="ExternalInput").ap()
    wvT = nc.dram_tensor("wvT", [128, KT_D * HG], F16, kind="ExternalInput").ap()
    bqv = nc.dram_tensor("bqv", [128, 4], F32, kind="ExternalInput").ap()
    bvb = nc.dram_tensor("bvb", [128, HG], F32, kind="ExternalInput").ap()
    woT = nc.dram_tensor("woT", [128, 2 * D], F16, kind="ExternalInput").ap()
    bob = nc.dram_tensor("bob", [128, KT_D], F32, kind="ExternalInput").ap()
    outT = nc.dram_tensor("outT", [128, QC * KT_D * 512], F16,
                          kind="ExternalOutput").ap()
    import os as _os
    _DBG = _os.environ.get("KDBG") == "1"
    if _DBG:
        dbg_q = nc.dram_tensor("dbg_q", [128, 2 * S], F16,
                               kind="ExternalOutput").ap()
        dbg_k = nc.dram_tensor("dbg_k", [128, 2 * S], F16,
                               kind="ExternalOutput").ap()
        dbg_vs = nc.dram_tensor("dbg_vs", [128, KT_S * 512], F16,
                                kind="ExternalOutput").ap()
        dbg_ao = nc.dram_tensor("dbg_ao", [128, 2 * S], F16,
                                kind="ExternalOutput").ap()

    wq_re = wqT.rearrange("p (kt c) -> p kt c", kt=KT_D)     # [128,8,256]
    wk_re = wkT.rearrange("p (kt c) -> p kt c", kt=KT_D)
    wv_re = wvT.rearrange("p (kt c) -> p kt c", kt=KT_D)
    wo_re = woT.rearrange("p (k2 c) -> p k2 c", k2=2)        # [128,2,1024]
    out_re = outT.rearrange("p (qc ot c) -> p qc ot c", qc=QC, ot=KT_D)

    with tile.TileContext(nc) as tc:
        with (
            tc.tile_pool(name="const", bufs=1) as cpool,
            tc.tile_pool(name="proj", bufs=1) as ppool,
            tc.tile_pool(name="xs", bufs=3) as xpool,
            tc.tile_pool(name="exp", bufs=6) as epool,
            tc.tile_pool(name="nrm", bufs=2) as npool,
            tc.tile_pool(name="ost", bufs=2) as opool,
            tc.tile_pool(name="psC", bufs=1, space="PSUM") as psC,
        ):
            # ---- persistent tiles ----
            wq_t = cpool.tile([128, KT_D, HG], F16)
            wk_t = cpool.tile([128, KT_D, HG], F16)
            wv_t = cpool.tile([128, KT_D, HG], F16)
            bqv_t = cpool.tile([128, 4], F32)
            bvb_t = cpool.tile([128, HG], F32)
            bob_t = cpool.tile([128, KT_D], F32)
            wo_t = cpool.tile([128, 2, D], F16)
            xv_all = cpool.tile([128, KT_D, S], F16)  # full V input, 4 MB

            qT = ppool.tile([128, 2, S], F16)  # [o-part, Mtile, t]
            kT = ppool.tile([128, 2, S], F16)
            vS = ppool.tile([128, KT_S, 4 * 128], F16)  # [t-part, t-tile, head*65]
            aoT = ppool.tile([128, 2, S], F16)  # normalized attn out^T

            # weight + V-input prefetch on the scalar HWDGE ring so the
            # x stream (sync ring) isn't blocked behind them
            nc.scalar.dma_start(wq_t[:], wq_re[:])
            nc.scalar.dma_start(bqv_t[:], bqv[:])
            nc.scalar.dma_start(wk_t[:], wk_re[:])
            nc.scalar.dma_start(wv_t[:], wv_re[:])
            nc.scalar.dma_start(bvb_t[:], bvb[:])
            nc.scalar.dma_start(wo_t[:], wo_re[:])
            nc.scalar.dma_start(bob_t[:], bob[:])
            nc.scalar.dma_start(
                xv_all[:, 0:4, :],
                xvT[0:128, :].rearrange("p (j c) -> p j c", j=4))
            nc.scalar.dma_start(
                xv_all[:, 4:8, :],
                xvT[128:256, :].rearrange("p (j c) -> p j c", j=4))

            # ---- phase A: Q^T then K^T projections (fp16, all-PSUM) ----
            # 8 chains: m0 -> two [128,1024] sc tiles (qc pairs), m1 -> four
            # [128,512] tiles in av0/av1/dD.
            for (w_t, w_dst, x_src, xtag, boff) in (
                (wq_t, qT, xqT, "xq_c", 0),
                (wk_t, kT, xkT, "xk_c", 2),
            ):
                sc01 = psC.tile([128, 1024], F32, name="sc01", tag="sc", bufs=2)
                sc23 = psC.tile([128, 1024], F32, name="sc23", tag="sc", bufs=2)
                m1c = [
                    psC.tile([128, 512], F32, name=f"m1c{qc}",
                             tag=("av0", "av1", "dD", "dD")[qc],
                             bufs=(1, 1, 2, 2)[qc])
                    for qc in range(QC)
                ]
                m0c = [sc01[:, 0:512], sc01[:, 512:1024],
                       sc23[:, 0:512], sc23[:, 512:1024]]
                for kc in range(KT_D // 2):
                    x2 = xpool.tile([128, 2, S], F16, name="x2", tag=xtag)
                    nc.sync.dma_start(
                        x2[:],
                        x_src[kc * 128:(kc + 1) * 128, :].rearrange(
                            "p (j c) -> p j c", j=2))
                    for jj in range(2):
                        kt = 2 * kc + jj
                        for qc in range(QC):
                            nc.tensor.matmul(
                                m0c[qc],
                                w_t[:, kt, 0:128],
                                x2[:, jj, qc * 512:(qc + 1) * 512],
                                start=(kt == 0), stop=(kt == KT_D - 1))
                        for qc in range(QC):
                            nc.tensor.matmul(
                                m1c[qc][:],
                                w_t[:, kt, 128:256],
                                x2[:, jj, qc * 512:(qc + 1) * 512],
                                start=(kt == 0), stop=(kt == KT_D - 1))
                nc.scalar.activation(
                    w_dst[:, 0, 0:1024], sc01[:], AF.Identity,
                    bias=bqv_t[:, boff:boff + 1])
                nc.scalar.activation(
                    w_dst[:, 0, 1024:2048], sc23[:], AF.Identity,
                    bias=bqv_t[:, boff:boff + 1])
                for qc in range(QC):
                    nc.scalar.activation(
                        w_dst[:, 1, qc * 512:(qc + 1) * 512], m1c[qc][:],
                        AF.Identity, bias=bqv_t[:, boff + 1:boff + 2])

            # ---- phase B: V projection (token-major layout, fp16) ----
            # ones columns of vS (softmax denominator trick): fill whole
            # tile with 1.0 once; V writes below overwrite all but col 64.
            nc.gpsimd.memset(vS[:], 1.0)
            for tg in range(QC):
                psv = [
                    psC.tile([128, HG], F32, name=f"psv{t}",
                             tag=("av0", "av1", "dD", "dD")[t],
                             bufs=(1, 1, 2, 2)[t])
                    for t in range(4)
                ]
                for kt in range(KT_D):
                    for t in range(4):
                        nc.tensor.matmul(
                            psv[t][:],
                            xv_all[:, kt, tg * 512 + t * 128:
                                   tg * 512 + (t + 1) * 128],
                            wv_t[:, kt, :], start=(kt == 0),
                            stop=(kt == KT_D - 1))
                for t in range(4):
                    tt = tg * 4 + t
                    for h in range(4):
                        nc.vector.tensor_tensor(
                            vS[:, tt, h * 128:h * 128 + 64],
                            psv[t][:, h * 64:(h + 1) * 64],
                            bvb_t[:, h * 64:(h + 1) * 64],
                            op=mybir.AluOpType.add)

            # ---- phase C: attention flat pipeline + interleaved out-proj ----
            state = {}

            def av_mms(st, kt):
                for i in range(2):
                    nc.tensor.matmul(
                        st["av"][i][:],
                        vS[:, kt, (2 * st["p"] + i) * 128:
                           (2 * st["p"] + i + 1) * 128],
                        st["ex"][kt][:, i * 512:(i + 1) * 512],
                        start=(kt == 0), stop=(kt == KT_S - 1))

            def normalize(st):
                p, qc = st["p"], st["qc"]
                for i in range(2):
                    uo = npool.tile([64, 512], F32, name="uo", tag=f"uo{i}")
                    nc.vector.tensor_copy(uo[:], st["av"][i][0:64, :])
                    sr = npool.tile([1, 512], F32, name="sr", tag=f"sr{i}")
                    nc.vector.tensor_copy(sr[:], st["av"][i][64:65, :])
                    rc = npool.tile([1, 512], F32, name="rc", tag=f"rc{i}")
                    scr = npool.tile([1, 512], F32, name="scr", tag=f"scr{i}")
                    nc.vector.reciprocal_approx_accurate(rc[:], sr[:], scr[:])
                    rb = npool.tile([64, 512], F32, name="rb", tag=f"rb{i}")
                    nc.gpsimd.partition_broadcast(rb[:], rc[:])
                    nc.vector.tensor_tensor(
                        aoT[i * 64:(i + 1) * 64, p, qc * 512:(qc + 1) * 512],
                        uo[:], rb[:], op=mybir.AluOpType.mult)

            def emit_D_pair(qc, ot0):
                for ot in (ot0, ot0 + 1):
                    acc2 = psC.tile([128, 512], F32, name=f"acc2{ot}",
                                    tag="dD", bufs=2)
                    for k2 in range(2):
                        nc.tensor.matmul(
                            acc2[:],
                            wo_t[:, k2, ot * 128:(ot + 1) * 128],
                            aoT[:, k2, qc * 512:(qc + 1) * 512],
                            start=(k2 == 0), stop=(k2 == 1))
                    nc.vector.tensor_scalar_add(
                        state["o_big"][:, ot, :], acc2[:], bob_t[:, ot:ot + 1])

            prev = None
            for qcp in range(2 * QC):
                qc, p = qcp // 2, qcp % 2
                cur = {"qc": qc, "p": p, "ex": [], "av": None}
                do_D = (p == 0 and qc > 0)
                for kt in range(KT_S):
                    sc = psC.tile([128, 1024], F32, name="sc", tag="sc",
                                  bufs=2)
                    nc.tensor.matmul(
                        sc[:, 0:512],
                        kT[0:64, p, kt * 128:(kt + 1) * 128],
                        qT[0:64, p, qc * 512:(qc + 1) * 512],
                        start=True, stop=True, tile_position=(0, 0))
                    nc.tensor.matmul(
                        sc[:, 512:1024],
                        kT[64:128, p, kt * 128:(kt + 1) * 128],
                        qT[64:128, p, qc * 512:(qc + 1) * 512],
                        start=True, stop=True, tile_position=(64, 0))
                    ex = epool.tile([128, 1024], F16, name="ex", tag="ex")
                    nc.scalar.activation(ex[:], sc[:], AF.Exp, scale=0.125)
                    cur["ex"].append(ex)

                    # previous iteration's AV tail + normalize, after this
                    # iteration's first scores so the exp stream never gaps
                    if kt == 0 and prev is not None:
                        av_mms(prev, KT_S - 2)
                    if kt == 1:
                        if prev is not None:
                            av_mms(prev, KT_S - 1)
                            normalize(prev)
                        cur["av"] = [
                            psC.tile([128, 512], F32, name=f"av{i}",
                                     tag=f"av{i}", bufs=1)
                            for i in range(2)
                        ]
                    # interleave previous token chunk's output projection
                    # (aoT of (qc-1, p1) finishes ~5us into this iteration)
                    if do_D:
                        if kt == 6:
                            state["o_big"] = opool.tile(
                                [128, KT_D, 512], F16, name="o_big",
                                tag="o_big")
                            emit_D_pair(qc - 1, 0)
                        elif kt == 9:
                            emit_D_pair(qc - 1, 2)
                        elif kt == 12:
                            emit_D_pair(qc - 1, 4)
                        elif kt == 15:
                            emit_D_pair(qc - 1, 6)
                    if kt >= 2:
                        av_mms(cur, kt - 2)
                if do_D:
                    nc.sync.dma_start(
                        out_re[:, qc - 1, :, :], state["o_big"][:])
                prev = cur

            # tail: last iteration's AV + normalize + last chunk's out-proj
            av_mms(prev, KT_S - 2)
            av_mms(prev, KT_S - 1)
            normalize(prev)
            state["o_big"] = opool.tile(
                [128, KT_D, 512], F16, name="o_big", tag="o_big")
            for ot0 in (0, 2, 4, 6):
                emit_D_pair(QC - 1, ot0)
            nc.sync.dma_start(out_re[:, QC - 1, :, :], state["o_big"][:])
            if _DBG:
                nc.sync.dma_start(
                    dbg_q.rearrange("p (m s) -> p m s", m=2), qT[:])
                nc.sync.dma_start(
                    dbg_k.rearrange("p (m s) -> p m s", m=2), kT[:])
                nc.sync.dma_start(
                    dbg_vs.rearrange("p (t c) -> p t c", t=KT_S), vS[:])
                nc.sync.dma_start(
                    dbg_ao.rearrange("p (m s) -> p m s", m=2), aoT[:])

    nc.compile()
    return nc


_NC = None


def _get_nc():
    global _NC
    if _NC is None:
        _NC = build_nc()
    return _NC


def _swz(a, groups, inner):
    """[groups*inner*128, C] -> [groups*128, inner*C] partition-major."""
    g, i = groups, inner
    rows, C = a.shape
    return np.ascontiguousarray(
        a.reshape(g, i, 128, C).transpose(0, 2, 1, 3).reshape(g * 128, i * C))


def kernel(q, k, v, Wq, bq, Wk, bk, Wv, bv, Wo, bo):
    nc = _get_nc()

    q = np.asarray(q, np.float32)
    k = np.asarray(k, np.float32)
    v = np.asarray(v, np.float32)

    xT = {}
    for b in range(B):
        xT[("q", b)] = _swz(np.ascontiguousarray(q[b].T).astype(np.float16), 4, 2)
        xT[("k", b)] = _swz(np.ascontiguousarray(k[b].T).astype(np.float16), 4, 2)
        xT[("v", b)] = _swz(np.ascontiguousarray(v[b].T).astype(np.float16), 2, 4)

    WqT = np.asarray(Wq, np.float32).T.astype(np.float16)
    WkT = np.asarray(Wk, np.float32).T.astype(np.float16)
    WvT = np.asarray(Wv, np.float32).T.astype(np.float16)
    WoT = np.asarray(Wo, np.float32).T.astype(np.float16)
    bq = np.asarray(bq, np.float32)
    bk = np.asarray(bk, np.float32)
    bv = np.asarray(bv, np.float32)
    bo = np.asarray(bo, np.float32)

    in_maps = []
    for c in range(NC_):
        b, g = divmod(c, 4)
        sl = slice(g * HG, (g + 1) * HG)
        bqs, bks = bq[sl], bk[sl]
        bqv_a = np.stack(
            [bqs[0:128], bqs[128:256], bks[0:128], bks[128:256]], axis=1)
        bo_a = (bo if g == 0 else np.zeros_like(bo)).reshape(KT_D, 128).T
        in_maps.append({
            "xqT": xT[("q", b)],
            "xkT": xT[("k", b)],
            "xvT": xT[("v", b)],
            "wqT": _swz(np.ascontiguousarray(WqT[:, sl]), 1, KT_D),
            "wkT": _swz(np.ascontiguousarray(WkT[:, sl]), 1, KT_D),
            "wvT": _swz(np.ascontiguousarray(WvT[:, sl]), 1, KT_D),
            "bqv": np.ascontiguousarray(bqv_a),
            "bvb": np.ascontiguousarray(
                np.broadcast_to(bv[sl], (128, HG))),
            "woT": _swz(np.ascontiguousarray(WoT[sl, :]), 1, 2),
            "bob": np.ascontiguousarray(bo_a),
        })

    res = run_bass_kernel_spmd(nc, in_maps, list(range(NC_)))

    out = np.empty((B, S, D), np.float32)
    for b in range(B):
        acc = np.zeros((128, QC, KT_D, 512), np.float32)
        for g in range(4):
            acc += res.results[b * 4 + g]["outT"].astype(np.float32).reshape(
                128, QC, KT_D, 512)
        # [p, qc, ot, c] -> [ot*128+p, qc*512+c] = out^T [D, S]
        oT = acc.transpose(2, 0, 1, 3).reshape(D, S)
        out[b] = oT.T
    return out


# revision 14
# speedup vs baseline: 1.3604x; 1.1017x over previous
"""Trainium2 multi-head attention kernel (8 NeuronCores).

Sharding: 2 (batch) x 4 (head-group) grid. Core c handles batch b=c//4 and
heads [4g, 4g+4) where g=c%4 (d_model slice of 256).

All-fp16 datapath (PSUM accumulates fp32), fully software-pipelined around
the ScalarE exp stream (~1016ns per [128,1024] tile), which is the binding
engine floor together with the PE:

  head (~15us): weights + first 1MB x chunks land, Q/K token-chunk 0
  projected, V token-group 0 projected, a dummy exp preloads the ACT table.
  phase C: 8 exp-paced iterations of scores (fp16 row-packed pairs) ->
  exp -> AV (lag 3). The remaining K chunks, V token-groups, and Q chunks
  are injected into early iterations' PE slack through the dD PSUM ring
  (epilogues on DVE), while their input DMAs stream under C. The output
  projection of each finished token chunk is interleaved at kt 6/9/12/15.

PSUM (8 banks): sc [128,1024] x2 (4) + av0/av1 [128,512] x1 (2) +
dD [128,512] x2 (2). Normalize copies av out through one [65,512] DVE copy
so the AV bank frees ~0.7us after the last AV matmul (enables av bufs=1).
"""
import sys

import numpy as np

for _p in ("/opt/trn_rl_repo", "/root/.axon_site/_ro/trn_rl_repo"):
    if _p not in sys.path:
        sys.path.append(_p)

import concourse.bacc as bacc
import concourse.mybir as mybir
import concourse.tile as tile
from concourse.bass_utils import run_bass_kernel_spmd

F32 = mybir.dt.float32
F16 = mybir.dt.float16

B, S, D, H, DK = 2, 2048, 1024, 16, 64
NC_ = 8
HG = D // 4          # 256: d_model slice per core
KT_D = D // 128      # 8 contraction tiles for projections
KT_S = S // 128      # 16 sequence tiles
QC = S // 512        # 4 query chunks of 512
LAG = 3              # AV lags exp by 3 kt
AF = mybir.ActivationFunctionType


def build_nc():
    nc = bacc.Bacc("TRN2", target_bir_lowering=False, debug=False, num_devices=NC_)

    # x tensors token-chunk swizzled: row cc*128+p, col kt*512+c
    xqT = nc.dram_tensor("xqT", [4 * 128, KT_D * 512], F16, kind="ExternalInput").ap()
    xkT = nc.dram_tensor("xkT", [4 * 128, KT_D * 512], F16, kind="ExternalInput").ap()
    xvT = nc.dram_tensor("xvT", [4 * 128, KT_D * 512], F16, kind="ExternalInput").ap()
    wqT = nc.dram_tensor("wqT", [128, KT_D * HG], F16, kind="ExternalInput").ap()
    wkT = nc.dram_tensor("wkT", [128, KT_D * HG], F16, kind="ExternalInput").ap()
    wvT = nc.dram_tensor("wvT", [128, KT_D * HG], F16, kind="ExternalInput").ap()
    bqv = nc.dram_tensor("bqv", [128, 4], F32, kind="ExternalInput").ap()
    bvb = nc.dram_tensor("bvb", [128, HG], F32, kind="ExternalInput").ap()
    woT = nc.dram_tensor("woT", [128, 2 * D], F16, kind="ExternalInput").ap()
    bob = nc.dram_tensor("bob", [128, KT_D], F32, kind="ExternalInput").ap()
    outT = nc.dram_tensor("outT", [128, QC * KT_D * 512], F16,
                          kind="ExternalOutput").ap()

    wq_re = wqT.rearrange("p (kt c) -> p kt c", kt=KT_D)
    wk_re = wkT.rearrange("p (kt c) -> p kt c", kt=KT_D)
    wv_re = wvT.rearrange("p (kt c) -> p kt c", kt=KT_D)
    wo_re = woT.rearrange("p (k2 c) -> p k2 c", k2=2)
    out_re = outT.rearrange("p (qc ot c) -> p qc ot c", qc=QC, ot=KT_D)

    def xchunk(x, cc):
        return x[cc * 128:(cc + 1) * 128, :].rearrange(
            "p (kt c) -> p kt c", kt=KT_D)

    with tile.TileContext(nc) as tc:
        with (
            tc.tile_pool(name="const", bufs=1) as cpool,
            tc.tile_pool(name="proj", bufs=1) as ppool,
            tc.tile_pool(name="exp", bufs=7) as epool,
            tc.tile_pool(name="nrm", bufs=1) as npool,
            tc.tile_pool(name="ost", bufs=2) as opool,
            tc.tile_pool(name="psC", bufs=1, space="PSUM") as psC,
        ):
            # ---- persistent tiles ----
            wq_t = cpool.tile([128, KT_D, HG], F16)
            wk_t = cpool.tile([128, KT_D, HG], F16)
            wv_t = cpool.tile([128, KT_D, HG], F16)
            bqv_t = cpool.tile([128, 4], F32)
            bvb_t = cpool.tile([128, HG], F32)
            bob_t = cpool.tile([128, KT_D], F32)
            wo_t = cpool.tile([128, 2, D], F16)
            dmy = cpool.tile([1, 2], F32)
            xq_all = cpool.tile([128, QC, KT_D, 512], F16)
            xk_all = cpool.tile([128, QC, KT_D, 512], F16)
            xv_all = cpool.tile([128, QC, KT_D, 512], F16)

            qT = ppool.tile([128, 2, S], F16)  # [o-part, Mtile, t]
            kT = ppool.tile([128, 2, S], F16)
            vS = ppool.tile([128, KT_S, 4 * 128], F16)  # [t-part, t-tile, head*65]
            aoT = ppool.tile([128, 2, S], F16)

            # ---- DMA streams ----
            # sync ring: x chunks in deadline order; scalar ring: weights
            nc.sync.dma_start(xq_all[:, 0], xchunk(xqT, 0))
            nc.sync.dma_start(xk_all[:, 0], xchunk(xkT, 0))
            nc.sync.dma_start(xv_all[:, 0], xchunk(xvT, 0))
            nc.sync.dma_start(xk_all[:, 1], xchunk(xkT, 1))
            nc.sync.dma_start(xv_all[:, 1], xchunk(xvT, 1))
            nc.sync.dma_start(xk_all[:, 2], xchunk(xkT, 2))
            nc.sync.dma_start(xv_all[:, 2], xchunk(xvT, 2))
            nc.sync.dma_start(xk_all[:, 3], xchunk(xkT, 3))
            nc.sync.dma_start(xv_all[:, 3], xchunk(xvT, 3))
            nc.sync.dma_start(xq_all[:, 1], xchunk(xqT, 1))
            nc.sync.dma_start(xq_all[:, 2], xchunk(xqT, 2))
            nc.sync.dma_start(xq_all[:, 3], xchunk(xqT, 3))
            nc.scalar.dma_start(wq_t[:], wq_re[:])
            nc.scalar.dma_start(bqv_t[:], bqv[:])
            nc.scalar.dma_start(wk_t[:], wk_re[:])
            nc.scalar.dma_start(wv_t[:], wv_re[:])
            nc.scalar.dma_start(bvb_t[:], bvb[:])
            nc.scalar.dma_start(wo_t[:], wo_re[:])
            nc.scalar.dma_start(bob_t[:], bob[:])

            # preload the exp ACT table off the critical path
            nc.gpsimd.memset(dmy[:], 0.0)
            nc.scalar.activation(dmy[0:1, 1:2], dmy[0:1, 0:1], AF.Exp)
            nc.gpsimd.memset(vS[:], 1.0)

            # ---- projection chunk emitters ----
            def qk_chunk(w_t, dst, x_all, cc, bcol, on_vector):
                """Project one 512-token chunk of Q or K (both Mtiles)."""
                t = psC.tile([128, 1024], F32, name=f"pjc{cc}", tag="sc",
                             bufs=2)
                for kt in range(KT_D):
                    for m in range(2):
                        nc.tensor.matmul(
                            t[:, m * 512:(m + 1) * 512],
                            w_t[:, kt, m * 128:(m + 1) * 128],
                            x_all[:, cc, kt, :],
                            start=(kt == 0), stop=(kt == KT_D - 1))
                for m in range(2):
                    o = dst[:, m, cc * 512:(cc + 1) * 512]
                    i_ = t[:, m * 512:(m + 1) * 512]
                    b = bqv_t[:, bcol + m:bcol + m + 1]
                    if on_vector:
                        nc.vector.tensor_scalar_add(o, i_, b)
                    else:
                        nc.scalar.activation(o, i_, AF.Identity, bias=b)

            def qk_chunk_steps(w_t, dst, x_all, cc, bcol):
                """Split chunk into 4 pacing steps (4 MMs each) + DVE epi.

                The PSUM tile is allocated inside step 0 so its dD-ring
                position matches emission order (alloc at build time would
                deadlock against V chains emitted in between)."""
                hold = {}

                def step(s):
                    if s == 0:
                        hold["t"] = [
                            psC.tile([128, 512], F32, name=f"pji{cc}{m}",
                                     tag="dD", bufs=2)
                            for m in range(2)
                        ]
                    t = hold["t"]
                    for kt in range(2 * s, 2 * s + 2):
                        for m in range(2):
                            nc.tensor.matmul(
                                t[m][:],
                                w_t[:, kt, m * 128:(m + 1) * 128],
                                x_all[:, cc, kt, :],
                                start=(kt == 0), stop=(kt == KT_D - 1))
                    if s == 3:
                        for m in range(2):
                            nc.vector.tensor_scalar_add(
                                dst[:, m, cc * 512:(cc + 1) * 512],
                                t[m][:],
                                bqv_t[:, bcol + m:bcol + m + 1])
                return [lambda s=s: step(s) for s in range(4)]

            def v_tchain(tg, t):
                """One [128 tokens] V projection chain + vS writes."""
                ps = psC.tile([128, HG], F32, name=f"psv{tg}{t}",
                              tag=("av0", "av1", "dD", "dD")[t] if tg == 0
                              else "dD",
                              bufs=(1, 1, 2, 2)[t] if tg == 0 else 2)
                for kt in range(KT_D):
                    nc.tensor.matmul(
                        ps[:],
                        xv_all[:, tg, kt, t * 128:(t + 1) * 128],
                        wv_t[:, kt, :], start=(kt == 0),
                        stop=(kt == KT_D - 1))
                tt = tg * 4 + t
                for h in range(4):
                    nc.vector.tensor_tensor(
                        vS[:, tt, h * 128:h * 128 + 64],
                        ps[:, h * 64:(h + 1) * 64],
                        bvb_t[:, h * 64:(h + 1) * 64],
                        op=mybir.AluOpType.add)

            # ---- head: Q/K chunk 0, K chunk 1, V token-groups 0-1 ----
            # emission order matches the x DMA arrival order
            qk_chunk(wq_t, qT, xq_all, 0, 0, on_vector=False)
            qk_chunk(wk_t, kT, xk_all, 0, 2, on_vector=False)
            for t in range(4):
                v_tchain(0, t)
            qk_chunk(wk_t, kT, xk_all, 1, 2, on_vector=False)
            for t in range(4):
                v_tchain(1, t)

            # ---- injection schedule: (qcp, kt) -> emitters ----
            # dD-ring users must be emitted in strict sequential order
            # (an alloc may only wait on releases of earlier-emitted work):
            # iter0: K-c2 kt0-3, V-tg2 kt4-7, K-c3 kt8-11, V-tg3 kt12-15.
            from collections import defaultdict
            inj = defaultdict(list)
            for j, base in ((2, 0), (3, 8)):
                steps = qk_chunk_steps(wk_t, kT, xk_all, j, 2)
                for s in range(4):
                    inj[(0, base + s)].append(steps[s])
            for g, base in ((2, 4), (3, 12)):
                for t in range(4):
                    inj[(0, base + t)].append(lambda g=g, t=t: v_tchain(g, t))
            # iters 1-3: Q chunks 1-3 at kt0-3 (before that iter's D pairs)
            for j in (1, 2, 3):
                steps = qk_chunk_steps(wq_t, qT, xq_all, j, 0)
                for s in range(4):
                    inj[(j, s)].append(steps[s])

            # ---- phase C ----
            state = {}

            def av_mms(st, kt):
                for i in range(2):
                    nc.tensor.matmul(
                        st["av"][i][:],
                        vS[:, kt, (2 * st["p"] + i) * 128:
                           (2 * st["p"] + i + 1) * 128],
                        st["ex"][kt][:, i * 512:(i + 1) * 512],
                        start=(kt == 0), stop=(kt == KT_S - 1))

            def normalize(st):
                p, qc = st["p"], st["qc"]
                for i in range(2):
                    uo = npool.tile([65, 512], F32, name="uo", tag=f"uo{i}")
                    nc.vector.tensor_copy(uo[:], st["av"][i][0:65, :])
                    sr = npool.tile([1, 512], F32, name="sr", tag=f"sr{i}")
                    nc.vector.tensor_copy(sr[:], uo[64:65, :])
                    rc = npool.tile([1, 512], F32, name="rc", tag=f"rc{i}")
                    scr = npool.tile([1, 512], F32, name="scr", tag=f"scr{i}")
                    nc.vector.reciprocal_approx_accurate(rc[:], sr[:], scr[:])
                    rb = npool.tile([64, 512], F32, name="rb", tag=f"rb{i}")
                    nc.gpsimd.partition_broadcast(rb[:], rc[:])
                    nc.vector.tensor_tensor(
                        aoT[i * 64:(i + 1) * 64, p, qc * 512:(qc + 1) * 512],
                        uo[0:64, :], rb[:], op=mybir.AluOpType.mult)

            def emit_D_pair(qc, ot0):
                for ot in (ot0, ot0 + 1):
                    acc2 = psC.tile([128, 512], F32, name=f"acc2{ot}",
                                    tag="dD", bufs=2)
                    for k2 in range(2):
                        nc.tensor.matmul(
                            acc2[:],
                            wo_t[:, k2, ot * 128:(ot + 1) * 128],
                            aoT[:, k2, qc * 512:(qc + 1) * 512],
                            start=(k2 == 0), stop=(k2 == 1))
                    nc.vector.tensor_scalar_add(
                        state["o_big"][:, ot, :], acc2[:], bob_t[:, ot:ot + 1])

            prev = None
            for qcp in range(2 * QC):
                qc, p = qcp // 2, qcp % 2
                cur = {"qc": qc, "p": p, "ex": [], "av": None}
                do_D = (p == 0 and qc > 0)
                for kt in range(KT_S):
                    sc = psC.tile([128, 1024], F32, name="sc", tag="sc",
                                  bufs=2)
                    nc.tensor.matmul(
                        sc[:, 0:512],
                        kT[0:64, p, kt * 128:(kt + 1) * 128],
                        qT[0:64, p, qc * 512:(qc + 1) * 512],
                        start=True, stop=True, tile_position=(0, 0))
                    nc.tensor.matmul(
                        sc[:, 512:1024],
                        kT[64:128, p, kt * 128:(kt + 1) * 128],
                        qT[64:128, p, qc * 512:(qc + 1) * 512],
                        start=True, stop=True, tile_position=(64, 0))
                    ex = epool.tile([128, 1024], F16, name="ex", tag="ex")
                    nc.scalar.activation(ex[:], sc[:], AF.Exp, scale=0.125)
                    cur["ex"].append(ex)

                    # previous iteration's AV tail + normalize, after this
                    # iteration's scores so the exp stream never gaps
                    if prev is not None and kt < LAG - 1:
                        av_mms(prev, KT_S - LAG + kt)
                    if kt == LAG - 1:
                        if prev is not None:
                            av_mms(prev, KT_S - 1)
                            normalize(prev)
                        cur["av"] = [
                            psC.tile([128, 512], F32, name=f"av{i}",
                                     tag=f"av{i}", bufs=1)
                            for i in range(2)
                        ]
                    for fn in inj.get((qcp, kt), ()):
                        fn()
                    if do_D:
                        if kt == 6:
                            state["o_big"] = opool.tile(
                                [128, KT_D, 512], F16, name="o_big",
                                tag="o_big")
                            emit_D_pair(qc - 1, 0)
                        elif kt == 9:
                            emit_D_pair(qc - 1, 2)
                        elif kt == 12:
                            emit_D_pair(qc - 1, 4)
                        elif kt == 15:
                            emit_D_pair(qc - 1, 6)
                    if kt >= LAG:
                        av_mms(cur, kt - LAG)
                if do_D:
                    nc.sync.dma_start(
                        out_re[:, qc - 1, :, :], state["o_big"][:])
                prev = cur

            # tail
            for kt in range(KT_S - LAG, KT_S):
                av_mms(prev, kt)
            normalize(prev)
            state["o_big"] = opool.tile(
                [128, KT_D, 512], F16, name="o_big", tag="o_big")
            for ot0 in (0, 2, 4, 6):
                emit_D_pair(QC - 1, ot0)
            nc.sync.dma_start(out_re[:, QC - 1, :, :], state["o_big"][:])

    nc.compile()
    return nc


_NC = None


def _get_nc():
    global _NC
    if _NC is None:
        _NC = build_nc()
    return _NC


def _swz_w(a, inner):
    """[inner*128, C] -> [128, inner*C] partition-major."""
    rows, C = a.shape
    return np.ascontiguousarray(
        a.reshape(inner, 128, C).transpose(1, 0, 2).reshape(128, inner * C))


def _swz_tok(a):
    """[1024, 2048] -> [512, 4096]: row cc*128+p, col kt*512+c."""
    return np.ascontiguousarray(
        a.reshape(8, 128, 4, 512).transpose(2, 1, 0, 3).reshape(512, 4096))


def kernel(q, k, v, Wq, bq, Wk, bk, Wv, bv, Wo, bo):
    nc = _get_nc()

    q = np.asarray(q, np.float32)
    k = np.asarray(k, np.float32)
    v = np.asarray(v, np.float32)

    xT = {}
    for b in range(B):
        xT[("q", b)] = _swz_tok(np.ascontiguousarray(q[b].T).astype(np.float16))
        xT[("k", b)] = _swz_tok(np.ascontiguousarray(k[b].T).astype(np.float16))
        xT[("v", b)] = _swz_tok(np.ascontiguousarray(v[b].T).astype(np.float16))

    WqT = np.asarray(Wq, np.float32).T.astype(np.float16)
    WkT = np.asarray(Wk, np.float32).T.astype(np.float16)
    WvT = np.asarray(Wv, np.float32).T.astype(np.float16)
    WoT = np.asarray(Wo, np.float32).T.astype(np.float16)
    bq = np.asarray(bq, np.float32)
    bk = np.asarray(bk, np.float32)
    bv = np.asarray(bv, np.float32)
    bo = np.asarray(bo, np.float32)

    in_maps = []
    for c in range(NC_):
        b, g = divmod(c, 4)
        sl = slice(g * HG, (g + 1) * HG)
        bqs, bks = bq[sl], bk[sl]
        bqv_a = np.stack(
            [bqs[0:128], bqs[128:256], bks[0:128], bks[128:256]], axis=1)
        bo_a = (bo if g == 0 else np.zeros_like(bo)).reshape(KT_D, 128).T
        in_maps.append({
            "xqT": xT[("q", b)],
            "xkT": xT[("k", b)],
            "xvT": xT[("v", b)],
            "wqT": _swz_w(np.ascontiguousarray(WqT[:, sl]), KT_D),
            "wkT": _swz_w(np.ascontiguousarray(WkT[:, sl]), KT_D),
            "wvT": _swz_w(np.ascontiguousarray(WvT[:, sl]), KT_D),
            "bqv": np.ascontiguousarray(bqv_a),
            "bvb": np.ascontiguousarray(
                np.broadcast_to(bv[sl], (128, HG))),
            "woT": _swz_w(np.ascontiguousarray(WoT[sl, :]), 2),
            "bob": np.ascontiguousarray(bo_a),
        })

    res = run_bass_kernel_spmd(nc, in_maps, list(range(NC_)))

    out = np.empty((B, S, D), np.float32)
    for b in range(B):
        acc = np.zeros((128, QC, KT_D, 512), np.float32)
        for g in range(4):
            acc += res.results[b * 4 + g]["outT"].astype(np.float32).reshape(
                128, QC, KT_D, 512)
        oT = acc.transpose(2, 0, 1, 3).reshape(D, S)
        out[b] = oT.T
    return out
